# revision 12
# baseline (speedup 1.0000x reference)
"""Raw-bass (no Tile) pipelined TT-linear kernel.

Math: W (1024x1024) is a rank-20 TT product, so
  y = (x @ Hin) @ [Hout; bias] with Hin (1024,20), Hout (20,1024).
Data-parallel over batch: 8 cores x 2048 rows.

Per-core schedule (bf16 compute, PE at 1.2 GHz is the scarce resource):
  GEMM1 is column-packed: the PE array's four 32-column groups each run
  one k-chunk concurrently (tile_position=(0,32j)), two packs of 4
  accumulating into one PSUM bank -> partial sums P_j+P_{j+4} land at
  partitions 32j..32j+19. GEMM2 contracts K=128 against a 4x-replicated
  [Hout;bias;0-gaps] so no partial-sum merge is needed.

Engine roles:
  sync   : x-chunk input DMAs (chunk 0 split in halves)
  scalar : weight DMAs + most PSUM->SBUF y evacuations
  vector : t4 group copies + some y evacuations
  tensor : matmuls, software-pipelined G1(c+1) before G2(c)
  gpsimd : output DMAs + end-of-kernel semaphore cleanup
"""

from contextlib import ExitStack

import numpy as np

import concourse.bass as bass
import concourse.mybir as mybir
from concourse.bass_utils import run_bass_kernel_spmd

N_CORES = 8
B_SHARD = 2048
D_IN = 1024
D_OUT = 1024
R = 20
KC = 8
CHUNK = 512
N_CHUNKS = B_SHARD // CHUNK
BT = CHUNK // 128
QPC = 2 * BT  # half-tiles per chunk
P2_BUFS = 6
BIAS_ROW = 116  # gap row carrying the ones/bias trick

_DT = {"f32": mybir.dt.float32, "bf16": mybir.dt.bfloat16}


def _dve_half(q):
    """True if half-tile q is evacuated by VectorE (else ScalarE)."""
    return q % 8 in (0, 3, 6)


def _yv_count(q):
    """# of DVE-evacuated halves with index <= q."""
    return sum(1 for i in range(q + 1) if _dve_half(i))


def _ys_count(q):
    """# of ACT-evacuated halves with index <= q."""
    return (q + 1) - _yv_count(q)


def build_nc(compute="bf16", out_bf16=True):
    cdt = _DT[compute]
    odt = mybir.dt.bfloat16 if out_bf16 else mybir.dt.float32
    f32 = mybir.dt.float32

    nc = bass.Bass("TRN2", target_bir_lowering=False, debug=False)

    xt_d = nc.declare_dram_parameter(
        "xt", [N_CHUNKS, 128, KC * CHUNK], cdt, isOutput=False
    )
    hin_d = nc.declare_dram_parameter("hin", [128, KC * R], cdt, isOutput=False)
    houtb_d = nc.declare_dram_parameter("houtb", [128, D_OUT], cdt, isOutput=False)
    ones_d = nc.declare_dram_parameter("ones", [1, CHUNK], cdt, isOutput=False)
    out_d = nc.declare_dram_parameter("out", [B_SHARD, D_OUT], odt, isOutput=True)

    with ExitStack() as ctx:
        hin_sb = ctx.enter_context(nc.sbuf_tensor("hin_sb", [128, KC * R], cdt))
        houtb_sb = ctx.enter_context(
            nc.sbuf_tensor("houtb_sb", [128, D_OUT], cdt)
        )
        xt_sb = [
            ctx.enter_context(nc.sbuf_tensor(f"xt{i}", [128, KC * CHUNK], cdt))
            for i in range(N_CHUNKS)
        ]
        t4_sb = [
            ctx.enter_context(nc.sbuf_tensor(f"t4{i}", [128, CHUNK], cdt))
            for i in range(2)
        ]
        y_sb = [
            ctx.enter_context(nc.sbuf_tensor(f"y{i}", [128, BT * D_OUT], odt))
            for i in range(2)
        ]
        # all 4 column groups share one bank per chunk; DVE pre-zeroes the
        # bank and every GEMM1 matmul uses start=False (accumulate-onto-0
        # is correct whatever the has_written state), so no bank-wide
        # clear ever races a concurrent group
        # full-bank tiles (a shared half-bank would let PE writes and DVE
        # reads of different chunks collide fatally in one bank)
        p1 = [
            ctx.enter_context(nc.psum_tensor(f"p1{i}", [128, 512], f32))
            for i in range(2)
        ]
        p2 = [
            ctx.enter_context(nc.psum_tensor(f"p2_{i}", [128, 512], f32))
            for i in range(P2_BUFS)
        ]
        # One semaphore per DMA synchronization point: a dma_start's
        # then_inc(sem, 16) is 16 independent +1s (one per SDMA engine),
        # so only "all transfers on this sem are done" thresholds are
        # race-free (and per-engine FIFO then covers earlier transfers).
        sem_whin = ctx.enter_context(nc.semaphore("sem_whin"))
        sem_whoutb = ctx.enter_context(nc.semaphore("sem_whoutb"))
        sem_ones = ctx.enter_context(nc.semaphore("sem_ones"))
        sem_xtp = [
            ctx.enter_context(nc.semaphore(f"sem_xtp{i}")) for i in range(4)
        ]
        sem_xtc = [
            ctx.enter_context(nc.semaphore(f"sem_xtc{i}"))
            for i in range(N_CHUNKS)
        ]  # index 0 unused
        sem_outc = [
            ctx.enter_context(nc.semaphore(f"sem_outc{i}"))
            for i in range(N_CHUNKS)
        ]
        # compute semaphores (single +1s in program order — cumulative ok)
        (sem_mm1, sem_t4, sem_mm2, sem_yv, sem_ys, sem_ms, sem_p1z) = [
            ctx.enter_context(nc.semaphore(n))
            for n in (
                "sem_mm1", "sem_t4", "sem_mm2", "sem_yv", "sem_ys",
                "sem_ms", "sem_p1z",
            )
        ]
        sems = (
            [sem_whin, sem_whoutb, sem_ones]
            + sem_xtp
            + sem_xtc
            + sem_outc
            + [sem_mm1, sem_t4, sem_mm2, sem_yv, sem_ys, sem_ms, sem_p1z]
        )
        nums = sorted(s.num for s in sems)
        assert nums == list(range(nums[0], nums[0] + len(nums))), nums
        sem_range = range(nums[0], nums[-1] + 1)

        def evac_wait(engine, q):
            """Wait until evacuation of half-tile q has completed."""
            if _dve_half(q):
                engine.wait_ge(sem_yv, _yv_count(q))
            else:
                engine.wait_ge(sem_ys, _ys_count(q))

        with nc.Block() as block:

            @block.sync
            def _(sync):
                # weights lead on this ring: on the other HWDGE ring they
                # starve behind the 4MiB input stream (measured: houtb
                # landing at ~19us instead of ~10)
                sync.dma_start(out=hin_sb[:], in_=hin_d[:]).then_inc(
                    sem_whin, 16
                )
                sync.dma_start(out=houtb_sb[:], in_=houtb_d[:]).then_inc(
                    sem_whoutb, 16
                )
                sync.dma_start(out=xt_sb[0][:], in_=xt_d[0][:]).then_inc(
                    sem_xtp[0], 16
                )
                sync.wait_ge(sem_ms, 1)  # memset-zero before the ones rows
                for i in range(2):
                    sync.dma_start(
                        out=t4_sb[i][BIAS_ROW : BIAS_ROW + 1, :], in_=ones_d[:]
                    ).then_inc(sem_ones, 16)
                for c in range(1, N_CHUNKS):
                    sync.dma_start(out=xt_sb[c][:], in_=xt_d[c]).then_inc(
                        sem_xtc[c], 16
                    )

            @block.tensor
            def _(tensor):
                def g1(c):
                    # concurrent column-group matmuls into a DVE-pre-zeroed
                    # bank, all start=False; group = kc % 4. Chunk 0 is
                    # paced by quarter-arrivals (packs of 2), the rest run
                    # two packs of 4.
                    for kc in range(KC):
                        j = kc % 4
                        if kc == 0:
                            if c == 0:
                                tensor.wait_ge(sem_whin, 16)
                            tensor.wait_ge(sem_p1z, c + 1)
                        if c == 0:
                            if kc == 0:
                                tensor.wait_ge(sem_xtp[0], 16)
                        elif kc == 0:
                            tensor.wait_ge(sem_xtc[c], 16)
                        mm = tensor.matmul(
                            p1[c % 2][32 * j : 32 * j + R, 0:CHUNK],
                            hin_sb[:, kc * R : (kc + 1) * R],
                            xt_sb[c][:, kc * CHUNK : (kc + 1) * CHUNK],
                            start=False,
                            stop=(kc == KC - 1),
                            tile_position=(0, 32 * j),
                            skip_group_check=True,
                        )
                        if kc == KC - 1:
                            mm.then_inc(sem_mm1)

                def g2(c):
                    for bt in range(BT):
                        for nh in range(2):
                            q = QPC * c + 2 * bt + nh
                            if q == QPC * c:
                                if c == 0:
                                    tensor.wait_ge(sem_whoutb, 16)
                                    tensor.wait_ge(sem_ones, 32)
                                tensor.wait_ge(sem_t4, c + 1)
                            if q >= P2_BUFS:
                                evac_wait(tensor, q - P2_BUFS)
                            tensor.matmul(
                                p2[q % P2_BUFS][:],
                                t4_sb[c % 2][:, bt * 128 : (bt + 1) * 128],
                                houtb_sb[:, nh * 512 : (nh + 1) * 512],
                                start=True,
                                stop=True,
                            ).then_inc(sem_mm2)

                for c in range(N_CHUNKS):
                    g1(c)
                    g2(c)

            @block.vector
            def _(vector):
                # zero both t4 buffers once: gap rows stay 0 forever,
                # row 116 becomes the ones row (DMA'd on the scalar ring)
                vector.memset(t4_sb[0][:], 0.0)
                vector.memset(t4_sb[1][:], 0.0).then_inc(sem_ms)

                def p1zero(c):
                    # DVE FIFO puts this after t4copy(c-2), the bank's last
                    # reader; g1(c) waits sem_p1z >= c+1 before writing
                    vector.memset(p1[c % 2][:], 0.0).then_inc(sem_p1z)

                def t4copy(c):
                    vector.wait_ge(sem_mm1, c + 1)
                    if c >= 2:
                        # t4 buffer reuse: all GEMM2 of chunk c-2 done
                        vector.wait_ge(sem_mm2, QPC * (c - 2) + QPC)
                    vector.tensor_copy(
                        t4_sb[c % 2][0:BIAS_ROW, :],
                        p1[c % 2][0:BIAS_ROW, 0:CHUNK],
                    ).then_inc(sem_t4)

                def evacs(c):
                    for bt in range(BT):
                        for nh in range(2):
                            q = QPC * c + 2 * bt + nh
                            if not _dve_half(q):
                                continue
                            vector.wait_ge(sem_mm2, q + 1)
                            first_dve = next(
                                i for i in range(QPC * c, QPC * (c + 1))
                                if _dve_half(i)
                            )
                            if q == first_dve and c >= 2:
                                vector.wait_ge(
                                    sem_outc[c - 2], 16 * BT
                                )  # y buf reuse
                            o0 = bt * D_OUT + nh * 512
                            vector.tensor_copy(
                                y_sb[c % 2][:, o0 : o0 + 512],
                                p2[q % P2_BUFS][:],
                            ).then_inc(sem_yv)

                p1zero(0)
                p1zero(1)
                for c in range(N_CHUNKS):
                    t4copy(c)
                    evacs(c)
                    if c + 2 < N_CHUNKS:
                        p1zero(c + 2)

            @block.scalar
            def _(scalar):
                # dummy copy: pull the one-time ACT_TABLE_LOAD (~1.3us) into
                # the idle head instead of the first real evacuation
                scalar.wait_ge(sem_ms, 1)
                scalar.copy(y_sb[0][0:1, 0:32], t4_sb[0][0:1, 0:32])
                for c in range(N_CHUNKS):
                    for bt in range(BT):
                        for nh in range(2):
                            q = QPC * c + 2 * bt + nh
                            if _dve_half(q):
                                continue
                            scalar.wait_ge(sem_mm2, q + 1)
                            first_act = next(
                                i for i in range(QPC * c, QPC * (c + 1))
                                if not _dve_half(i)
                            )
                            if q == first_act and c >= 2:
                                scalar.wait_ge(
                                    sem_outc[c - 2], 16 * BT
                                )  # y buf reuse
                            o0 = bt * D_OUT + nh * 512
                            scalar.copy(
                                y_sb[c % 2][:, o0 : o0 + 512],
                                p2[q % P2_BUFS][:],
                            ).then_inc(sem_ys)

            @block.gpsimd
            def _(gpsimd):
                for c in range(N_CHUNKS):
                    for bt in range(BT):
                        q1 = QPC * c + 2 * bt + 1
                        yv_need = _yv_count(q1)
                        ys_need = _ys_count(q1)
                        if yv_need:
                            gpsimd.wait_ge(sem_yv, yv_need)
                        if ys_need:
                            gpsimd.wait_ge(sem_ys, ys_need)
                        r0 = c * CHUNK + bt * 128
                        gpsimd.dma_start(
                            out=out_d[r0 : r0 + 128, :],
                            in_=y_sb[c % 2][:, bt * D_OUT : (bt + 1) * D_OUT],
                        ).then_inc(sem_outc[c], 16)
                # last chunk's transfers done => (per-engine FIFO) all done
                gpsimd.wait_ge(sem_outc[N_CHUNKS - 1], 16 * BT)
                # leave semaphores clean for any re-execution
                gpsimd.dma_reset(sem_range)
                gpsimd.sem_clear(sem_range)

    return nc


def host_prep(x, cores, bias, np_dt):
    A = cores[0][0].astype(np.float64)
    for G in cores[1:4]:
        G = G.astype(np.float64)
        A = np.einsum("ir,rjs->ijs", A, G).reshape(-1, G.shape[2])
    H = cores[4].astype(np.float64)
    for G in cores[5:]:
        G = G.astype(np.float64)
        H = np.einsum("pNq,qnr->pNnr", H, G).reshape(H.shape[0], -1, G.shape[2])
    H = H.reshape(H.shape[0], -1)  # (20, 1024)

    hin = np.ascontiguousarray(
        A.reshape(KC, 128, R).transpose(1, 0, 2).reshape(128, KC * R)
    ).astype(np_dt)
    # Hout replicated into the four 32-row column groups + bias in row 20
    houtb = np.zeros((128, D_OUT), dtype=np.float64)
    for j in range(4):
        houtb[32 * j : 32 * j + R, :] = H
    houtb[BIAS_ROW, :] = bias.astype(np.float64)
    houtb = houtb.astype(np_dt)
    xt = np.ascontiguousarray(
        x.reshape(N_CORES, N_CHUNKS, CHUNK, KC, 128).transpose(0, 1, 4, 3, 2)
    ).astype(np_dt).reshape(N_CORES, N_CHUNKS, 128, KC * CHUNK)
    ones = np.ones((1, CHUNK), dtype=np_dt)
    return xt, hin, houtb, ones


_NC_CACHE = {}


def run(x, cores, bias, compute="bf16", out_bf16=True, trace=False):
    np_dt = np.dtype(mybir.dt.np(_DT[compute]))
    xt, hin, houtb, ones = host_prep(x, cores, bias, np_dt)
    key = (compute, out_bf16)
    if key not in _NC_CACHE:
        _NC_CACHE[key] = build_nc(compute, out_bf16)
    nc = _NC_CACHE[key]
    in_maps = [
        {"xt": xt[i], "hin": hin, "houtb": houtb, "ones": ones}
        for i in range(N_CORES)
    ]
    res = run_bass_kernel_spmd(nc, in_maps, list(range(N_CORES)), trace=trace)
    out = np.concatenate([res.results[i]["out"] for i in range(N_CORES)], axis=0)
    return out.astype(np.float32), res


def kernel(x, core0, core1, core2, core3, core4, core5, core6, core7, bias):
    cores = (core0, core1, core2, core3, core4, core5, core6, core7)
    out, _ = run(
        np.asarray(x, dtype=np.float32),
        [np.asarray(c, dtype=np.float32) for c in cores],
        np.asarray(bias, dtype=np.float32),
    )
    return out


# revision 13
# speedup vs baseline: 1.1309x; 1.1309x over previous
"""Raw-bass (no Tile) pipelined TT-linear kernel.

Math: W (1024x1024) is a rank-20 TT product, so
  y = (x @ Hin) @ [Hout; bias] with Hin (1024,20), Hout (20,1024).
Data-parallel over batch: 8 cores x 2048 rows.

Per-core schedule (bf16 compute, PE at 1.2 GHz is the scarce resource):
  GEMM1 is column-packed: the PE array's four 32-column groups each run
  one k-chunk concurrently (tile_position=(0,32j)), two packs of 4
  accumulating into one PSUM bank -> partial sums P_j+P_{j+4} land at
  partitions 32j..32j+19. GEMM2 contracts K=128 against a 4x-replicated
  [Hout;bias;0-gaps] so no partial-sum merge is needed.

Engine roles:
  sync   : x-chunk input DMAs (chunk 0 split in halves)
  scalar : weight DMAs + most PSUM->SBUF y evacuations
  vector : t4 group copies + some y evacuations
  tensor : matmuls, software-pipelined G1(c+1) before G2(c)
  gpsimd : output DMAs + end-of-kernel semaphore cleanup
"""

from contextlib import ExitStack

import numpy as np

import concourse.bass as bass
import concourse.mybir as mybir
from concourse.bass_utils import run_bass_kernel_spmd

N_CORES = 8
B_SHARD = 2048
D_IN = 1024
D_OUT = 1024
R = 20
KC = 8
CHUNK = 512
N_CHUNKS = B_SHARD // CHUNK
BT = CHUNK // 128
QPC = 2 * BT  # half-tiles per chunk
P2_BUFS = 6
BIAS_ROW = 116  # gap row carrying the ones/bias trick

_DT = {"f32": mybir.dt.float32, "bf16": mybir.dt.bfloat16}


def _dve_half(q):
    """True if half-tile q is evacuated by VectorE (else ScalarE)."""
    return q % 8 in (0, 3, 6)


def _yv_count(q):
    """# of DVE-evacuated halves with index <= q."""
    return sum(1 for i in range(q + 1) if _dve_half(i))


def _ys_count(q):
    """# of ACT-evacuated halves with index <= q."""
    return (q + 1) - _yv_count(q)


def build_nc(compute="bf16", out_bf16=True):
    cdt = _DT[compute]
    odt = mybir.dt.bfloat16 if out_bf16 else mybir.dt.float32
    f32 = mybir.dt.float32

    nc = bass.Bass("TRN2", target_bir_lowering=False, debug=False)

    xt_d = nc.declare_dram_parameter(
        "xt", [N_CHUNKS, 128, KC * CHUNK], cdt, isOutput=False
    )
    hin_d = nc.declare_dram_parameter("hin", [128, KC * R], cdt, isOutput=False)
    houtb_d = nc.declare_dram_parameter("houtb", [128, D_OUT], cdt, isOutput=False)
    ones_d = nc.declare_dram_parameter("ones", [1, CHUNK], cdt, isOutput=False)
    out_d = nc.declare_dram_parameter("out", [B_SHARD, D_OUT], odt, isOutput=True)

    with ExitStack() as ctx:
        hin_sb = ctx.enter_context(nc.sbuf_tensor("hin_sb", [128, KC * R], cdt))
        houtb_sb = ctx.enter_context(
            nc.sbuf_tensor("houtb_sb", [128, D_OUT], cdt)
        )
        xt_sb = [
            ctx.enter_context(nc.sbuf_tensor(f"xt{i}", [128, KC * CHUNK], cdt))
            for i in range(N_CHUNKS)
        ]
        t4_sb = [
            ctx.enter_context(nc.sbuf_tensor(f"t4{i}", [128, CHUNK], cdt))
            for i in range(2)
        ]
        y_sb = [
            ctx.enter_context(nc.sbuf_tensor(f"y{i}", [128, BT * D_OUT], odt))
            for i in range(2)
        ]
        # all 4 column groups share one bank per chunk; DVE pre-zeroes the
        # bank and every GEMM1 matmul uses start=False (accumulate-onto-0
        # is correct whatever the has_written state), so no bank-wide
        # clear ever races a concurrent group
        # full-bank tiles (a shared half-bank would let PE writes and DVE
        # reads of different chunks collide fatally in one bank)
        p1 = [
            ctx.enter_context(nc.psum_tensor(f"p1{i}", [128, 512], f32))
            for i in range(2)
        ]
        p2 = [
            ctx.enter_context(nc.psum_tensor(f"p2_{i}", [128, 512], f32))
            for i in range(P2_BUFS)
        ]
        # One semaphore per DMA synchronization point: a dma_start's
        # then_inc(sem, 16) is 16 independent +1s (one per SDMA engine),
        # so only "all transfers on this sem are done" thresholds are
        # race-free (and per-engine FIFO then covers earlier transfers).
        sem_whin = ctx.enter_context(nc.semaphore("sem_whin"))
        sem_whoutb = ctx.enter_context(nc.semaphore("sem_whoutb"))
        sem_ones = ctx.enter_context(nc.semaphore("sem_ones"))
        sem_xtp = [
            ctx.enter_context(nc.semaphore(f"sem_xtp{i}")) for i in range(4)
        ]
        sem_xtc = [
            ctx.enter_context(nc.semaphore(f"sem_xtc{i}"))
            for i in range(N_CHUNKS)
        ]  # index 0 unused
        sem_outc = [
            ctx.enter_context(nc.semaphore(f"sem_outc{i}"))
            for i in range(N_CHUNKS)
        ]
        # compute semaphores (single +1s in program order — cumulative ok)
        (sem_mm1, sem_t4, sem_mm2, sem_yv, sem_ys, sem_ms, sem_p1z) = [
            ctx.enter_context(nc.semaphore(n))
            for n in (
                "sem_mm1", "sem_t4", "sem_mm2", "sem_yv", "sem_ys",
                "sem_ms", "sem_p1z",
            )
        ]
        sems = (
            [sem_whin, sem_whoutb, sem_ones]
            + sem_xtp
            + sem_xtc
            + sem_outc
            + [sem_mm1, sem_t4, sem_mm2, sem_yv, sem_ys, sem_ms, sem_p1z]
        )
        nums = sorted(s.num for s in sems)
        assert nums == list(range(nums[0], nums[0] + len(nums))), nums
        sem_range = range(nums[0], nums[-1] + 1)

        def evac_wait(engine, q):
            """Wait until evacuation of half-tile q has completed."""
            if _dve_half(q):
                engine.wait_ge(sem_yv, _yv_count(q))
            else:
                engine.wait_ge(sem_ys, _ys_count(q))

        with nc.Block() as block:

            @block.sync
            def _(sync):
                # weights lead on this ring: on the other HWDGE ring they
                # starve behind the 4MiB input stream (measured: houtb
                # landing at ~19us instead of ~10)
                sync.dma_start(out=hin_sb[:], in_=hin_d[:]).then_inc(
                    sem_whin, 16
                )
                sync.dma_start(out=xt_sb[0][:], in_=xt_d[0][:]).then_inc(
                    sem_xtp[0], 16
                )
                # houtb isn't needed until GEMM2(0); let xt0 lead it
                sync.dma_start(out=houtb_sb[:], in_=houtb_d[:]).then_inc(
                    sem_whoutb, 16
                )
                sync.wait_ge(sem_ms, 1)  # memset-zero before the ones rows
                for i in range(2):
                    sync.dma_start(
                        out=t4_sb[i][BIAS_ROW : BIAS_ROW + 1, :], in_=ones_d[:]
                    ).then_inc(sem_ones, 16)
                for c in range(1, N_CHUNKS):
                    sync.dma_start(out=xt_sb[c][:], in_=xt_d[c]).then_inc(
                        sem_xtc[c], 16
                    )
                # odd output tiles ride this ring (idle after inputs) so
                # out-DMA issue isn't serialized on one emitter
                for c in range(N_CHUNKS):
                    for bt in range(BT):
                        if (c * BT + bt) % 2 == 0:
                            continue
                        q1 = QPC * c + 2 * bt + 1
                        yv_need = _yv_count(q1)
                        ys_need = _ys_count(q1)
                        if yv_need:
                            sync.wait_ge(sem_yv, yv_need)
                        if ys_need:
                            sync.wait_ge(sem_ys, ys_need)
                        r0 = c * CHUNK + bt * 128
                        sync.dma_start(
                            out=out_d[r0 : r0 + 128, :],
                            in_=y_sb[c % 2][:, bt * D_OUT : (bt + 1) * D_OUT],
                        ).then_inc(sem_outc[c], 16)

            @block.tensor
            def _(tensor):
                def g1(c):
                    # concurrent column-group matmuls into a DVE-pre-zeroed
                    # bank, all start=False; group = kc % 4. Chunk 0 is
                    # paced by quarter-arrivals (packs of 2), the rest run
                    # two packs of 4.
                    for kc in range(KC):
                        j = kc % 4
                        if kc == 0:
                            if c == 0:
                                tensor.wait_ge(sem_whin, 16)
                            tensor.wait_ge(sem_p1z, c + 1)
                        if c == 0:
                            if kc == 0:
                                tensor.wait_ge(sem_xtp[0], 16)
                        elif kc == 0:
                            tensor.wait_ge(sem_xtc[c], 16)
                        mm = tensor.matmul(
                            p1[c % 2][32 * j : 32 * j + R, 0:CHUNK],
                            hin_sb[:, kc * R : (kc + 1) * R],
                            xt_sb[c][:, kc * CHUNK : (kc + 1) * CHUNK],
                            start=False,
                            stop=(kc == KC - 1),
                            tile_position=(0, 32 * j),
                            skip_group_check=True,
                        )
                        if kc == KC - 1:
                            mm.then_inc(sem_mm1)

                def g2(c):
                    for bt in range(BT):
                        for nh in range(2):
                            q = QPC * c + 2 * bt + nh
                            if q == QPC * c:
                                if c == 0:
                                    tensor.wait_ge(sem_whoutb, 16)
                                    tensor.wait_ge(sem_ones, 32)
                                tensor.wait_ge(sem_t4, c + 1)
                            if q >= P2_BUFS:
                                evac_wait(tensor, q - P2_BUFS)
                            tensor.matmul(
                                p2[q % P2_BUFS][:],
                                t4_sb[c % 2][:, bt * 128 : (bt + 1) * 128],
                                houtb_sb[:, nh * 512 : (nh + 1) * 512],
                                start=True,
                                stop=True,
                            ).then_inc(sem_mm2)

                for c in range(N_CHUNKS):
                    g1(c)
                    g2(c)

            @block.vector
            def _(vector):
                # zero both t4 buffers once: gap rows stay 0 forever,
                # row 116 becomes the ones row (DMA'd on the scalar ring)
                vector.memset(t4_sb[0][:], 0.0)
                vector.memset(t4_sb[1][:], 0.0).then_inc(sem_ms)

                def p1zero(c):
                    # DVE FIFO puts this after t4copy(c-2), the bank's last
                    # reader; g1(c) waits sem_p1z >= c+1 before writing
                    vector.memset(p1[c % 2][:], 0.0).then_inc(sem_p1z)

                def t4copy(c):
                    vector.wait_ge(sem_mm1, c + 1)
                    if c >= 2:
                        # t4 buffer reuse: all GEMM2 of chunk c-2 done
                        vector.wait_ge(sem_mm2, QPC * (c - 2) + QPC)
                    vector.tensor_copy(
                        t4_sb[c % 2][0:BIAS_ROW, :],
                        p1[c % 2][0:BIAS_ROW, 0:CHUNK],
                    ).then_inc(sem_t4)

                def evacs(c):
                    for bt in range(BT):
                        for nh in range(2):
                            q = QPC * c + 2 * bt + nh
                            if not _dve_half(q):
                                continue
                            vector.wait_ge(sem_mm2, q + 1)
                            first_dve = next(
                                i for i in range(QPC * c, QPC * (c + 1))
                                if _dve_half(i)
                            )
                            if q == first_dve and c >= 2:
                                vector.wait_ge(
                                    sem_outc[c - 2], 16 * BT
                                )  # y buf reuse
                            o0 = bt * D_OUT + nh * 512
                            vector.tensor_copy(
                                y_sb[c % 2][:, o0 : o0 + 512],
                                p2[q % P2_BUFS][:],
                            ).then_inc(sem_yv)

                p1zero(0)
                p1zero(1)
                for c in range(N_CHUNKS):
                    t4copy(c)
                    evacs(c)
                    if c + 2 < N_CHUNKS:
                        p1zero(c + 2)

            @block.scalar
            def _(scalar):
                # dummy copy: pull the one-time ACT_TABLE_LOAD (~1.3us) into
                # the idle head instead of the first real evacuation
                scalar.wait_ge(sem_ms, 1)
                scalar.copy(y_sb[0][0:1, 0:32], t4_sb[0][0:1, 0:32])
                for c in range(N_CHUNKS):
                    for bt in range(BT):
                        for nh in range(2):
                            q = QPC * c + 2 * bt + nh
                            if _dve_half(q):
                                continue
                            scalar.wait_ge(sem_mm2, q + 1)
                            first_act = next(
                                i for i in range(QPC * c, QPC * (c + 1))
                                if not _dve_half(i)
                            )
                            if q == first_act and c >= 2:
                                scalar.wait_ge(
                                    sem_outc[c - 2], 16 * BT
                                )  # y buf reuse
                            o0 = bt * D_OUT + nh * 512
                            scalar.copy(
                                y_sb[c % 2][:, o0 : o0 + 512],
                                p2[q % P2_BUFS][:],
                            ).then_inc(sem_ys)

            @block.gpsimd
            def _(gpsimd):
                for c in range(N_CHUNKS):
                    for bt in range(BT):
                        if (c * BT + bt) % 2 == 1:
                            continue  # odd tiles go via the sync ring
                        q1 = QPC * c + 2 * bt + 1
                        yv_need = _yv_count(q1)
                        ys_need = _ys_count(q1)
                        if yv_need:
                            gpsimd.wait_ge(sem_yv, yv_need)
                        if ys_need:
                            gpsimd.wait_ge(sem_ys, ys_need)
                        r0 = c * CHUNK + bt * 128
                        gpsimd.dma_start(
                            out=out_d[r0 : r0 + 128, :],
                            in_=y_sb[c % 2][:, bt * D_OUT : (bt + 1) * D_OUT],
                        ).then_inc(sem_outc[c], 16)
                # last chunk's transfers done => (per-engine FIFO) all done
                gpsimd.wait_ge(sem_outc[N_CHUNKS - 1], 16 * BT)
                # leave semaphores clean for any re-execution
                gpsimd.dma_reset(sem_range)
                gpsimd.sem_clear(sem_range)

    return nc


def host_prep(x, cores, bias, np_dt):
    A = cores[0][0].astype(np.float64)
    for G in cores[1:4]:
        G = G.astype(np.float64)
        A = np.einsum("ir,rjs->ijs", A, G).reshape(-1, G.shape[2])
    H = cores[4].astype(np.float64)
    for G in cores[5:]:
        G = G.astype(np.float64)
        H = np.einsum("pNq,qnr->pNnr", H, G).reshape(H.shape[0], -1, G.shape[2])
    H = H.reshape(H.shape[0], -1)  # (20, 1024)

    hin = np.ascontiguousarray(
        A.reshape(KC, 128, R).transpose(1, 0, 2).reshape(128, KC * R)
    ).astype(np_dt)
    # Hout replicated into the four 32-row column groups + bias in row 20
    houtb = np.zeros((128, D_OUT), dtype=np.float64)
    for j in range(4):
        houtb[32 * j : 32 * j + R, :] = H
    houtb[BIAS_ROW, :] = bias.astype(np.float64)
    houtb = houtb.astype(np_dt)
    xt = np.ascontiguousarray(
        x.reshape(N_CORES, N_CHUNKS, CHUNK, KC, 128).transpose(0, 1, 4, 3, 2)
    ).astype(np_dt).reshape(N_CORES, N_CHUNKS, 128, KC * CHUNK)
    ones = np.ones((1, CHUNK), dtype=np_dt)
    return xt, hin, houtb, ones


_NC_CACHE = {}


def run(x, cores, bias, compute="bf16", out_bf16=True, trace=False):
    np_dt = np.dtype(mybir.dt.np(_DT[compute]))
    xt, hin, houtb, ones = host_prep(x, cores, bias, np_dt)
    key = (compute, out_bf16)
    if key not in _NC_CACHE:
        _NC_CACHE[key] = build_nc(compute, out_bf16)
    nc = _NC_CACHE[key]
    in_maps = [
        {"xt": xt[i], "hin": hin, "houtb": houtb, "ones": ones}
        for i in range(N_CORES)
    ]
    res = run_bass_kernel_spmd(nc, in_maps, list(range(N_CORES)), trace=trace)
    out = np.concatenate([res.results[i]["out"] for i in range(N_CORES)], axis=0)
    return out.astype(np.float32), res


def kernel(x, core0, core1, core2, core3, core4, core5, core6, core7, bias):
    cores = (core0, core1, core2, core3, core4, core5, core6, core7)
    out, _ = run(
        np.asarray(x, dtype=np.float32),
        [np.asarray(c, dtype=np.float32) for c in cores],
        np.asarray(bias, dtype=np.float32),
    )
    return out
